# revision 1
# baseline (speedup 1.0000x reference)
"""Trainium2 Bass kernel for the CodedNet shift-mask-reduce problem.

Math (from the reference):
    out[b, i, j] = sum_c x[b, i, j, c] * bk[(i - c) % 256, j, c]

Strategy:
  - Host: fuse the two rolls into a per-channel roll of the mask
    (M[i, c, j] = bk[(i-c)%256, j, c]), transpose x to [b, i, c, j] so the
    channel axis is a middle free axis on-chip, cast both to fp16 (the mask
    is exactly 0/1 so products are exact; only x quantization + tree-add
    rounding remain, ~7e-4 absmax-relative). The rolled mask has period 128 in both i and j for
    this problem's tiled-2x2 mask (verified at run time), so only a
    [128, 28, 128] slab is shipped, shared by both i-blocks, with the j
    axis broadcast on-chip via a stride-0 AP.
  - Shard batch 32 -> 4 per NeuronCore across 8 cores (pure data parallel).
  - Device, per core: tiles cover (batch, i-block) [128(i) x 28(c) x 256(j)];
    DVE tensor_mul with the mask broadcast over j-halves, then a binary
    add-tree over the channel axis (28 -> 14 -> 7 -> 4 -> 2 -> 1), final add
    emits f32. All big ops are 16-bit SBUF step-1 -> DVE 2x perf mode.
    Tiles 0/1 load+multiply in channel chunks so compute starts after
    ~0.7 MB of DMA; tiles 2..7 run as 2-tile pairs whose small tree ops are
    batched over the pair axis to halve per-op init overhead; the last
    pair's finals/stores are split to shorten the tail.
  - Gather: concat per-core outputs on the batch axis.
"""

import os

import numpy as np
import ml_dtypes

B, P, C = 32, 256, 28
N_CORES = 8
B_PER_CORE = B // N_CORES  # 4
N_TILES = B_PER_CORE * 2  # (b, iblk)

# "f16" (fast, ~7e-4 absmax-rel) | "bf16" (fast, ~6e-3) | "f32" (slow fallback)
DTYPE = os.environ.get("KERNEL_DTYPE", "f16")
if DTYPE not in ("f16", "bf16", "f32"):
    DTYPE = "f16"
X_BUFS = int(os.environ.get("KERNEL_X_BUFS", "3"))
O_BUFS = int(os.environ.get("KERNEL_O_BUFS", "3"))

_CACHE = {}
LAST_RESULTS = None  # stash of BassKernelResults for profiling from test harness


def _build(dtype_str=None, x_bufs=None, o_bufs=None, split_first=True):
    dtype_str = dtype_str or DTYPE
    x_bufs = x_bufs or (2 if dtype_str == "f32" else X_BUFS)
    o_bufs = o_bufs or O_BUFS
    key = (dtype_str, x_bufs, o_bufs, split_first)
    if key in _CACHE:
        return _CACHE[key]

    import concourse.mybir as mybir
    from concourse import bacc, tile

    dt = {
        "bf16": mybir.dt.bfloat16,
        "f16": mybir.dt.float16,
        "f32": mybir.dt.float32,
    }[dtype_str]
    f32 = mybir.dt.float32

    nc = bacc.Bacc(
        "TRN2", target_bir_lowering=False, debug=False, num_devices=N_CORES
    )

    xt = nc.dram_tensor("xt", [N_TILES, 128, C, P], dt, kind="ExternalInput")
    mk = nc.dram_tensor("mk", [128, C, 128], dt, kind="ExternalInput")
    out = nc.dram_tensor("out", [N_TILES, 128, P], f32, kind="ExternalOutput")

    xt_ap, mk_ap, out_ap = xt.ap(), mk.ap(), out.ap()

    with tile.TileContext(nc) as tc:
        with (
            tc.tile_pool(name="mask", bufs=1) as mpool,
            tc.tile_pool(name="xs", bufs=2) as xspool,
            tc.tile_pool(name="x", bufs=x_bufs) as xpool,
            tc.tile_pool(name="o", bufs=o_bufs) as opool,
        ):
            v = nc.vector

            def mul(x_t, m, c_lo, c_hi):
                # x view [128, c, 2, 128]; mask broadcast over the j-half axis
                xv = x_t[:, c_lo:c_hi, :].rearrange(
                    "p c (h j) -> p c h j", h=2
                )
                mv = m[:, c_lo:c_hi, :].unsqueeze(2).broadcast_to(
                    [128, c_hi - c_lo, 2, 128]
                )
                v.tensor_mul(out=xv, in0=xv, in1=mv)

            def tree(x_t, t, quartered=False):
                if quartered == "gapfill":
                    # same total add-cycles as the plain tree, but the first
                    # ops depend on single channel-quarters, so they can run
                    # inside the DVE stalls while later quarter DMAs land.
                    # Channel coverage (each exactly once, verified):
                    #   A: {0,1,2}+={3,4,5}   B: {7,8,9}+={10,11,12}
                    #   C: [0:3]+=[7:10]      D: [6]+=[13]
                    #   E: [14:21]+=[21:28]   F: [14:17]+=[17:20]
                    #   G: [0:3]+=[14:17]     H: [6]+=[20]
                    #   I: [0]+=[1]  J: [0]+=[2]  K(f32): [0]+[6]
                    v.tensor_add(out=x_t[:, 0:3, :], in0=x_t[:, 0:3, :], in1=x_t[:, 3:6, :])
                    v.tensor_add(out=x_t[:, 7:10, :], in0=x_t[:, 7:10, :], in1=x_t[:, 10:13, :])
                    v.tensor_add(out=x_t[:, 0:3, :], in0=x_t[:, 0:3, :], in1=x_t[:, 7:10, :])
                    v.tensor_add(out=x_t[:, 6:7, :], in0=x_t[:, 6:7, :], in1=x_t[:, 13:14, :])
                    v.tensor_add(out=x_t[:, 14:21, :], in0=x_t[:, 14:21, :], in1=x_t[:, 21:28, :])
                    v.tensor_add(out=x_t[:, 14:17, :], in0=x_t[:, 14:17, :], in1=x_t[:, 17:20, :])
                    v.tensor_add(out=x_t[:, 0:3, :], in0=x_t[:, 0:3, :], in1=x_t[:, 14:17, :])
                    v.tensor_add(out=x_t[:, 6:7, :], in0=x_t[:, 6:7, :], in1=x_t[:, 20:21, :])
                    v.tensor_add(out=x_t[:, 0:1, :], in0=x_t[:, 0:1, :], in1=x_t[:, 1:2, :])
                    v.tensor_add(out=x_t[:, 0:1, :], in0=x_t[:, 0:1, :], in1=x_t[:, 2:3, :])
                    o = opool.tile([128, 1, P], f32)
                    v.tensor_add(out=o[:], in0=x_t[:, 0:1, :], in1=x_t[:, 6:7, :])
                    nc.scalar.dma_start(out=out_ap[t], in_=o[:, 0, :])
                    return
                if quartered:
                    # same total work, but the first two adds only need the
                    # first/second half of the channel quarters -> they can
                    # run while later quarter DMAs are still in flight
                    v.tensor_add(out=x_t[:, 0:7, :], in0=x_t[:, 0:7, :], in1=x_t[:, 7:14, :])
                    v.tensor_add(out=x_t[:, 14:21, :], in0=x_t[:, 14:21, :], in1=x_t[:, 21:28, :])
                    v.tensor_add(out=x_t[:, 0:7, :], in0=x_t[:, 0:7, :], in1=x_t[:, 14:21, :])
                else:
                    # channel add-tree: 28 -> 14 -> 7 -> 4 -> 2 -> 1
                    v.tensor_add(out=x_t[:, 0:14, :], in0=x_t[:, 0:14, :], in1=x_t[:, 14:28, :])
                    v.tensor_add(out=x_t[:, 0:7, :], in0=x_t[:, 0:7, :], in1=x_t[:, 7:14, :])
                v.tensor_add(out=x_t[:, 0:3, :], in0=x_t[:, 0:3, :], in1=x_t[:, 4:7, :])
                v.tensor_add(out=x_t[:, 0:2, :], in0=x_t[:, 0:2, :], in1=x_t[:, 2:4, :])
                o = opool.tile([128, 1, P], f32)
                v.tensor_add(out=o[:], in0=x_t[:, 0:1, :], in1=x_t[:, 1:2, :])
                nc.scalar.dma_start(out=out_ap[t], in_=o[:, 0, :])

            def pair_tail(p_t, t0, split_store=False, from7=False):
                # levels 2..5 of the channel tree, batched over the 2-tile
                # axis (the small ops are init-overhead-dominated; batching
                # halves their count while muls/L1 stay per-tile for overlap).
                # from7: halves already reduced to the 7-channel state.
                if not from7:
                    v.tensor_add(out=p_t[:, :, 0:7, :], in0=p_t[:, :, 0:7, :], in1=p_t[:, :, 7:14, :])
                v.tensor_add(out=p_t[:, :, 0:3, :], in0=p_t[:, :, 0:3, :], in1=p_t[:, :, 4:7, :])
                v.tensor_add(out=p_t[:, :, 0:2, :], in0=p_t[:, :, 0:2, :], in1=p_t[:, :, 2:4, :])
                if split_store:
                    # last pair: per-tile finals + stores so the very last
                    # store (on the critical tail) is half-sized
                    for q in (0, 1):
                        o = opool.tile([128, 1, P], f32, tag="osplit")
                        v.tensor_add(
                            out=o[:], in0=p_t[:, q, 0:1, :], in1=p_t[:, q, 1:2, :]
                        )
                        eng = nc.sync if q == 1 else nc.scalar
                        eng.dma_start(out=out_ap[t0 + q], in_=o[:, 0, :])
                else:
                    o = opool.tile([128, 2, 1, P], f32)
                    v.tensor_add(out=o[:], in0=p_t[:, :, 0:1, :], in1=p_t[:, :, 1:2, :])
                    nc.scalar.dma_start(
                        out=out_ap[t0 : t0 + 2].rearrange("t p j -> p t j"),
                        in_=o[:, :, 0, :],
                    )

            # --- ramp-optimized prologue: interleave mask/x channel chunks so
            # the first multiply starts after ~0.7 MB of DMA, not ~6.5 MB.
            # Tiles 0/1 run individually with channel-chunked loads (fast
            # ramp); tiles 2..7 run as 2-tile pairs sharing one op set
            # (halves the per-op init overhead).
            m0 = mpool.tile([128, C, 128], dt)
            if split_first:
                splits = {0: [0, 7, 14, 21, 28], 1: [0, 7, 14, 28]}
                for t in (0, 1):
                    cuts = splits[t]
                    x_t = xspool.tile([128, C, P], dt)
                    for lo, hi in zip(cuts[:-1], cuts[1:]):
                        if t == 0:
                            nc.sync.dma_start(
                                out=m0[:, lo:hi, :], in_=mk_ap[:, lo:hi, :]
                            )
                        nc.sync.dma_start(
                            out=x_t[:, lo:hi, :], in_=xt_ap[t, :, lo:hi, :]
                        )
                    for lo, hi in zip(cuts[:-1], cuts[1:]):
                        mul(x_t, m0, lo, hi)
                    tree(x_t, t, quartered=(t == 0))
                start = 2
            else:
                nc.sync.dma_start(out=m0[:], in_=mk_ap[:])
                start = 0

            for t0 in range(start, N_TILES, 2):
                p_t = xpool.tile([128, 2, C, P], dt)
                for q in (0, 1):
                    nc.sync.dma_start(out=p_t[:, q], in_=xt_ap[t0 + q])
                    mul(p_t[:, q], m0, 0, C)
                    # L1 per half so it can start as soon as its load lands
                    v.tensor_add(
                        out=p_t[:, q, 0:14, :],
                        in0=p_t[:, q, 0:14, :],
                        in1=p_t[:, q, 14:28, :],
                    )
                pair_tail(p_t, t0, split_store=(t0 + 2 >= N_TILES))

    nc.compile()
    _CACHE[key] = nc
    return nc


def _prep_mask(bk, np_dt):
    """M[i, c, j] = bk[(i-c)%P, j, c]; return [2, 128, C, 128] slab if the
    rolled mask is 128-periodic in i and j (always true for this problem's
    tiled-2x2, channel-repeated mask), else None. Shape [128, C, 128]."""
    M = np.empty((P, C, P), dtype=np.float32)
    for c in range(C):
        M[:, c, :] = np.roll(bk[:, :, c], c, axis=0)
    per_i = np.array_equal(M[:128], M[128:])
    per_j = np.array_equal(M[:, :, :128], M[:, :, 128:])
    if not (per_i and per_j):
        return None
    return np.ascontiguousarray(M[:128, :, :128].astype(np_dt))


def kernel(x: np.ndarray, bk: np.ndarray) -> np.ndarray:
    global LAST_RESULTS
    from concourse.bass_utils import run_bass_kernel_spmd

    x = np.asarray(x, dtype=np.float32)
    bk = np.asarray(bk, dtype=np.float32)
    np_dt = {"bf16": ml_dtypes.bfloat16, "f16": np.float16, "f32": np.float32}[
        DTYPE
    ]

    mk = _prep_mask(bk, np_dt)
    if mk is None:
        # generic fallback: no periodicity -> full-j mask, plain jax-free numpy
        return _kernel_generic(x, bk)

    # x [B, i, j, c] -> [B, i, c, j], cast, shard per core
    xc = x.astype(np_dt)  # contiguous cast first (fast), then strided copy at 2B
    xt = np.ascontiguousarray(xc.transpose(0, 1, 3, 2))  # [B, 256, C, P]
    xt = xt.reshape(N_CORES, N_TILES, 128, C, P)

    nc = _build()
    in_maps = [{"xt": xt[k], "mk": mk} for k in range(N_CORES)]
    res = run_bass_kernel_spmd(nc, in_maps, core_ids=list(range(N_CORES)))
    LAST_RESULTS = res

    outs = [res.results[k]["out"].reshape(B_PER_CORE, P, P) for k in range(N_CORES)]
    return np.concatenate(outs, axis=0)


def _kernel_generic(x: np.ndarray, bk: np.ndarray) -> np.ndarray:
    """Safety net for a non-periodic mask: plain numpy (never taken for the
    real problem inputs, whose mask is tiled 2x2 and channel-repeated)."""
    M = np.empty((P, C, P), dtype=np.float32)
    for c in range(C):
        M[:, c, :] = np.roll(bk[:, :, c], c, axis=0)
    # out[b,i,j] = sum_c x[b,i,j,c] * M[i,c,j]
    return np.einsum("bijc,icj->bij", x.astype(np.float32), M, optimize=True).astype(
        np.float32
    )



# revision 5
# speedup vs baseline: 1.3081x; 1.3081x over previous
"""Trainium2 Bass kernel for the CodedNet shift-mask-reduce problem.

Math (from the reference):
    out[b, i, j] = sum_c x[b, i, j, c] * bk[(i - c) % 256, j, c]

Architecture (v2 — DVE multiply + PE selection-reduce):
  - Host: fuse the two rolls into the mask W[i', c, j'] = bk[(i'-c)%256, j', c],
    which is 128-periodic in both i and j for this problem's tiled-2x2,
    channel-repeated mask (verified at run time; generic numpy fallback
    otherwise). Cast x to f16.
  - Layout: SBUF partitions carry (c, g) = 28 channels x 4 i-groups = 112
    rows; free axis = (i_sub in [0,32), i1 in {0,1}, j in [0,256)), where
    i = i1*128 + 32*g + i_sub. Host pre-permutes x into this layout.
  - Per (core, batch) block:
      * DMA x chunks [112, 8, 2, 256] f16,
      * DVE tensor_mul by the mask slab (2x perf mode; mask broadcast over
        i1 and j-halves via stride-0 dims),
      * PE: 32 accumulating "selection matmuls" into PSUM [128, 2, 256]:
        pass p uses stationary S_p[(c,g), m] = 1 iff m == 32g + p (a sliding
        128-wide slice of one [112, 160] 0/1 matrix), rhs = y[:, p, :, :].
        This performs the 28-channel sums on the TensorEngine, with the
        cost-model matmul cost depending only on output columns.
      * Act engine drains PSUM f32 -> SBUF f16; store f16 (halves out DMA).
  - Shard batch 32 -> 4 per NeuronCore across 8 cores (pure data parallel).
  - Host: final [b, i', i1, j] f16 -> [b, i, j] f32.
"""

import numpy as np

B, P, C = 32, 256, 28
N_CORES = 8
B_PER_CORE = B // N_CORES  # 4
G = 4          # i-groups per pass -> partitions = C * G = 112
NPART = C * G  # 112
ISUB = P // 2 // G  # 32 i_sub values per group
NPASS = ISUB   # 32 PE passes per block
CHUNK = 8      # i_sub values per DMA/mul chunk

DTYPE = "f16"  # informational (test.py prints it)
_CACHE = {}
LAST_RESULTS = None  # stash of BassKernelResults for profiling from test harness


def _build():
    key = "v2"
    if key in _CACHE:
        return _CACHE[key]

    import concourse.mybir as mybir
    from concourse import bacc, tile

    f16 = mybir.dt.float16
    f32 = mybir.dt.float32

    nc = bacc.Bacc(
        "TRN2", target_bir_lowering=False, debug=False, num_devices=N_CORES
    )

    xt = nc.dram_tensor(
        "xt", [B_PER_CORE, NPART, ISUB, 2, P], f16, kind="ExternalInput"
    )
    mk = nc.dram_tensor("mk", [NPART, ISUB, 128], f16, kind="ExternalInput")
    em = nc.dram_tensor("em", [NPART, 160], f16, kind="ExternalInput")
    out = nc.dram_tensor("out", [B_PER_CORE, 128, 2, P], f16, kind="ExternalOutput")

    xt_ap, mk_ap, em_ap, out_ap = xt.ap(), mk.ap(), em.ap(), out.ap()

    with tile.TileContext(nc) as tc:
        with (
            tc.tile_pool(name="sel", bufs=1) as spool,
            tc.tile_pool(name="mask", bufs=1) as mpool,
            tc.tile_pool(name="x", bufs=2) as xpool,
            tc.tile_pool(name="ps", bufs=2, space="PSUM") as ppool,
            tc.tile_pool(name="o", bufs=2) as opool,
        ):
            v = nc.vector

            e_t = spool.tile([NPART, 160], f16, tag="e")
            nc.sync.dma_start(out=e_t[:], in_=em_ap)
            m_t = mpool.tile([NPART, ISUB, 128], f16, tag="m")

            for b in range(B_PER_CORE):
                y_t = xpool.tile([NPART, ISUB, 2, P], f16, tag="y")
                ps_t = ppool.tile([128, 2, P], f32, tag="ps")
                for s0 in range(0, ISUB, CHUNK):
                    s1 = s0 + CHUNK
                    if b == 0:
                        nc.sync.dma_start(
                            out=m_t[:, s0:s1, :], in_=mk_ap[:, s0:s1, :]
                        )
                    nc.sync.dma_start(
                        out=y_t[:, s0:s1], in_=xt_ap[b, :, s0:s1]
                    )
                    # y view [112, chunk, 4, j']; mask broadcast over (i1, h)
                    yv = y_t[:, s0:s1].rearrange("p s a (h j) -> p s (a h) j", h=2)
                    mv = (
                        m_t[:, s0:s1, :]
                        .unsqueeze(2)
                        .broadcast_to([NPART, CHUNK, 4, 128])
                    )
                    v.tensor_mul(out=yv, in0=yv, in1=mv)
                    for p in range(s0, s1):
                        nc.tensor.matmul(
                            out=ps_t[:],
                            lhsT=e_t[:, 31 - p : 159 - p],
                            rhs=y_t[:, p],
                            start=(p == 0),
                            stop=(p == NPASS - 1),
                        )
                o_t = opool.tile([128, 2, P], f16, tag="o")
                nc.scalar.copy(out=o_t[:], in_=ps_t[:])
                nc.scalar.dma_start(out=out_ap[b], in_=o_t[:])

    nc.compile()
    _CACHE[key] = nc
    return nc


def _prep_mask(bk):
    """W[i', c, j'] = bk[(i'-c)%P, j', c]; verify 128-periodicity in i and j,
    then return the (c,g)-layout slab [112, 32, 128] f16, or None."""
    M = np.empty((P, C, P), dtype=np.float32)
    for c in range(C):
        M[:, c, :] = np.roll(bk[:, :, c], c, axis=0)
    if not (
        np.array_equal(M[:128], M[128:])
        and np.array_equal(M[:, :, :128], M[:, :, 128:])
    ):
        return None
    W = M[:128, :, :128]  # [i', c, j']
    # [i'=32g+i_sub, c, j'] -> [c, g, i_sub, j'] -> [112, 32, 128]
    slab = W.reshape(G, ISUB, C, 128).transpose(2, 0, 1, 3).reshape(NPART, ISUB, 128)
    return np.ascontiguousarray(slab.astype(np.float16))


def _sel_matrix():
    E = np.zeros((NPART, 160), dtype=np.float16)
    for c in range(C):
        for g in range(G):
            E[c * G + g, 32 * g + 31] = 1.0
    return E


def kernel(x: np.ndarray, bk: np.ndarray) -> np.ndarray:
    global LAST_RESULTS
    from concourse.bass_utils import run_bass_kernel_spmd

    x = np.asarray(x, dtype=np.float32)
    bk = np.asarray(bk, dtype=np.float32)

    mk = _prep_mask(bk)
    if mk is None:
        return _kernel_generic(x, bk)

    # x [B, i, j, c] -> [core, b, c, g, i_sub, i1, j] with i = i1*128+32g+i_sub
    xc = x.astype(np.float16)
    xc = xc.reshape(N_CORES, B_PER_CORE, 2, G, ISUB, P, C)
    xt = np.ascontiguousarray(xc.transpose(0, 1, 6, 3, 4, 2, 5))
    xt = xt.reshape(N_CORES, B_PER_CORE, NPART, ISUB, 2, P)

    em = _sel_matrix()

    nc = _build()
    in_maps = [{"xt": xt[k], "mk": mk, "em": em} for k in range(N_CORES)]
    res = run_bass_kernel_spmd(nc, in_maps, core_ids=list(range(N_CORES)))
    LAST_RESULTS = res

    # out [b, i'(128), i1, j] f16 -> [b, i, j] f32
    outs = [
        res.results[k]["out"].transpose(0, 2, 1, 3).reshape(B_PER_CORE, P, P)
        for k in range(N_CORES)
    ]
    return np.concatenate(outs, axis=0).astype(np.float32)


def _kernel_generic(x: np.ndarray, bk: np.ndarray) -> np.ndarray:
    """Safety net for a non-periodic mask: plain numpy (never taken for the
    real problem inputs, whose mask is tiled 2x2 and channel-repeated)."""
    M = np.empty((P, C, P), dtype=np.float32)
    for c in range(C):
        M[:, c, :] = np.roll(bk[:, :, c], c, axis=0)
    return np.einsum("bijc,icj->bij", x.astype(np.float32), M, optimize=True).astype(
        np.float32
    )


# revision 7
# speedup vs baseline: 1.3411x; 1.0252x over previous
"""Trainium2 Bass kernel for the CodedNet shift-mask-reduce problem.

Math (from the reference):
    out[b, i, j] = sum_c x[b, i, j, c] * bk[(i - c) % 256, j, c]

Architecture (v2 — DVE multiply + PE selection-reduce):
  - Host: fuse the two rolls into the mask W[i', c, j'] = bk[(i'-c)%256, j', c],
    which is 128-periodic in both i and j for this problem's tiled-2x2,
    channel-repeated mask (verified at run time; generic numpy fallback
    otherwise). Cast x to f16.
  - Layout: SBUF partitions carry (c, g) = 28 channels x 4 i-groups = 112
    rows; free axis = (i_sub in [0,32), i1 in {0,1}, j in [0,256)), where
    i = i1*128 + 32*g + i_sub. Host pre-permutes x into this layout.
  - Per (core, batch) block:
      * DMA x chunks [112, 8, 2, 256] f16,
      * DVE tensor_mul by the mask slab (2x perf mode; mask broadcast over
        i1 and j-halves via stride-0 dims),
      * PE: 32 accumulating "selection matmuls" into PSUM [128, 2, 256]:
        pass p uses stationary S_p[(c,g), m] = 1 iff m == 32g + p (a sliding
        128-wide slice of one [112, 160] 0/1 matrix), rhs = y[:, p, :, :].
        This performs the 28-channel sums on the TensorEngine, with the
        cost-model matmul cost depending only on output columns.
      * Act engine drains PSUM f32 -> SBUF f16; store f16 (halves out DMA).
  - Shard batch 32 -> 4 per NeuronCore across 8 cores (pure data parallel).
  - Host: final [b, i', i1, j] f16 -> [b, i, j] f32.
"""

import numpy as np

B, P, C = 32, 256, 28
N_CORES = 8
B_PER_CORE = B // N_CORES  # 4
G = 4          # i-groups per pass -> partitions = C * G = 112
NPART = C * G  # 112
ISUB = P // 2 // G  # 32 i_sub values per group
NPASS = ISUB   # 32 PE passes per block
CHUNK = 8      # i_sub values per DMA/mul chunk

DTYPE = "f16"  # informational (test.py prints it)
_CACHE = {}
LAST_RESULTS = None  # stash of BassKernelResults for profiling from test harness


def _build():
    key = "v2"
    if key in _CACHE:
        return _CACHE[key]

    import concourse.mybir as mybir
    from concourse import bacc, tile

    f16 = mybir.dt.float16
    f32 = mybir.dt.float32

    nc = bacc.Bacc(
        "TRN2", target_bir_lowering=False, debug=False, num_devices=N_CORES
    )

    i8 = mybir.dt.int8

    xt = nc.dram_tensor(
        "xt", [B_PER_CORE, NPART, ISUB, 2, P], f16, kind="ExternalInput"
    )
    mk = nc.dram_tensor("mk", [NPART, ISUB, 128], i8, kind="ExternalInput")
    em = nc.dram_tensor("em", [NPART, 160], f16, kind="ExternalInput")
    out = nc.dram_tensor("out", [B_PER_CORE, 128, 2, P], f16, kind="ExternalOutput")

    xt_ap, mk_ap, em_ap, out_ap = xt.ap(), mk.ap(), em.ap(), out.ap()

    with tile.TileContext(nc) as tc:
        with (
            tc.tile_pool(name="sel", bufs=1) as spool,
            tc.tile_pool(name="mask", bufs=1) as mpool,
            tc.tile_pool(name="x", bufs=2) as xpool,
            tc.tile_pool(name="ps", bufs=2, space="PSUM") as ppool,
            tc.tile_pool(name="o", bufs=2) as opool,
        ):
            v = nc.vector

            e_t = spool.tile([NPART, 160], f16, tag="e")
            nc.sync.dma_start(out=e_t[:], in_=em_ap)
            m8_t = mpool.tile([NPART, ISUB, 128], i8, tag="m8")
            m_t = mpool.tile([NPART, ISUB, 128], f16, tag="m")

            for b in range(B_PER_CORE):
                # finer trailing chunks on the last block shorten the tail
                cuts = (
                    [0, 8, 16, 24, 28, 32]
                    if b == B_PER_CORE - 1
                    else [0, 8, 16, 24, 32]
                )
                y_t = xpool.tile([NPART, ISUB, 2, P], f16, tag="y")
                ps_t = ppool.tile([128, 2, P], f32, tag="ps")
                for s0, s1 in zip(cuts[:-1], cuts[1:]):
                    if b == 0:
                        nc.sync.dma_start(
                            out=m8_t[:, s0:s1, :], in_=mk_ap[:, s0:s1, :]
                        )
                        v.tensor_copy(out=m_t[:, s0:s1, :], in_=m8_t[:, s0:s1, :])
                    nc.sync.dma_start(
                        out=y_t[:, s0:s1], in_=xt_ap[b, :, s0:s1]
                    )
                    # y view [112, chunk, 4, j']; mask broadcast over (i1, h)
                    yv = y_t[:, s0:s1].rearrange("p s a (h j) -> p s (a h) j", h=2)
                    mv = (
                        m_t[:, s0:s1, :]
                        .unsqueeze(2)
                        .broadcast_to([NPART, s1 - s0, 4, 128])
                    )
                    v.tensor_mul(out=yv, in0=yv, in1=mv)
                    for p in range(s0, s1):
                        nc.tensor.matmul(
                            out=ps_t[:],
                            lhsT=e_t[:, 31 - p : 159 - p],
                            rhs=y_t[:, p],
                            start=(p == 0),
                            stop=(p == NPASS - 1),
                        )
                if b == B_PER_CORE - 1:
                    # split drain+store so the final dependent piece is small;
                    # last piece drains on DVE (lower fixed access cost)
                    o_t = opool.tile([128, 2, P], f16, tag="o")
                    nc.scalar.copy(out=o_t[:, 0], in_=ps_t[:, 0])
                    nc.scalar.dma_start(out=out_ap[b, :, 0], in_=o_t[:, 0])
                    v.tensor_copy(out=o_t[:, 1], in_=ps_t[:, 1])
                    nc.scalar.dma_start(out=out_ap[b, :, 1], in_=o_t[:, 1])
                else:
                    o_t = opool.tile([128, 2, P], f16, tag="o")
                    nc.scalar.copy(out=o_t[:], in_=ps_t[:])
                    nc.scalar.dma_start(out=out_ap[b], in_=o_t[:])

    nc.compile()
    _CACHE[key] = nc
    return nc


def _prep_mask(bk):
    """W[i', c, j'] = bk[(i'-c)%P, j', c]; verify 128-periodicity in i and j,
    then return the (c,g)-layout slab [112, 32, 128] f16, or None."""
    M = np.empty((P, C, P), dtype=np.float32)
    for c in range(C):
        M[:, c, :] = np.roll(bk[:, :, c], c, axis=0)
    if not (
        np.array_equal(M[:128], M[128:])
        and np.array_equal(M[:, :, :128], M[:, :, 128:])
    ):
        return None
    W = M[:128, :, :128]  # [i', c, j']
    # [i'=32g+i_sub, c, j'] -> [c, g, i_sub, j'] -> [112, 32, 128]
    slab = W.reshape(G, ISUB, C, 128).transpose(2, 0, 1, 3).reshape(NPART, ISUB, 128)
    return np.ascontiguousarray(slab.astype(np.int8))


def _sel_matrix():
    E = np.zeros((NPART, 160), dtype=np.float16)
    for c in range(C):
        for g in range(G):
            E[c * G + g, 32 * g + 31] = 1.0
    return E


def kernel(x: np.ndarray, bk: np.ndarray) -> np.ndarray:
    global LAST_RESULTS
    from concourse.bass_utils import run_bass_kernel_spmd

    x = np.asarray(x, dtype=np.float32)
    bk = np.asarray(bk, dtype=np.float32)

    mk = _prep_mask(bk)
    if mk is None:
        return _kernel_generic(x, bk)

    # x [B, i, j, c] -> [core, b, c, g, i_sub, i1, j] with i = i1*128+32g+i_sub
    xc = x.astype(np.float16)
    xc = xc.reshape(N_CORES, B_PER_CORE, 2, G, ISUB, P, C)
    xt = np.ascontiguousarray(xc.transpose(0, 1, 6, 3, 4, 2, 5))
    xt = xt.reshape(N_CORES, B_PER_CORE, NPART, ISUB, 2, P)

    em = _sel_matrix()

    nc = _build()
    in_maps = [{"xt": xt[k], "mk": mk, "em": em} for k in range(N_CORES)]
    res = run_bass_kernel_spmd(nc, in_maps, core_ids=list(range(N_CORES)))
    LAST_RESULTS = res

    # out [b, i'(128), i1, j] f16 -> [b, i, j] f32
    outs = [
        res.results[k]["out"].transpose(0, 2, 1, 3).reshape(B_PER_CORE, P, P)
        for k in range(N_CORES)
    ]
    return np.concatenate(outs, axis=0).astype(np.float32)


def _kernel_generic(x: np.ndarray, bk: np.ndarray) -> np.ndarray:
    """Safety net for a non-periodic mask: plain numpy (never taken for the
    real problem inputs, whose mask is tiled 2x2 and channel-repeated)."""
    M = np.empty((P, C, P), dtype=np.float32)
    for c in range(C):
        M[:, c, :] = np.roll(bk[:, :, c], c, axis=0)
    return np.einsum("bijc,icj->bij", x.astype(np.float32), M, optimize=True).astype(
        np.float32
    )
